# revision 36
# baseline (speedup 1.0000x reference)
"""MiniMaxText01 linear attention on 8 trn2 NeuronCores.

Sharding: core c -> batch b = c//4, head-quad g = c%4 (global heads 4g..4g+3).
Single merged pass over all 4 heads per core; out_proj is row-parallel so each
core emits one fp16 partial [S, HID]; the host sums 4 partials per batch.

All on-chip tensors live in transposed [feature, token] layout so every matmul
contraction sits on the partition dim. The qkv projection contracts K=2048 as
8 fp16 K-tiles + 8 e4m3 K-tiles (4 DoubleRow matmuls at 2x rate); operands are
pre-scaled by 16 (x) and 512 (w) on host and the product descaled for free via
the activation's scale argument. Gate/out projections and attention are fp16
with fp32 PSUM. Attention runs on 128-token chunks (mathematically identical
to the reference's 256 chunks, fewer FLOPs/token).
"""

import numpy as np

B, S, HID = 2, 4096, 2048
H, D = 16, 128
C = 128                # attention chunk
BLK = 512              # token block (4 chunks)
NBLK = S // BLK        # 8 blocks
KO = HID // 128        # 16 contraction subtiles
KO16 = 4               # k-tiles 0..3 in fp16
KO8 = 12               # k-tiles 4..15 in e4m3 (DoubleRow pairs)
NCORES = 8
P = 128
SX = 16.0              # x pre-scale (both halves)
SW = 512.0             # w_qkv/wv pre-scale (both halves)

_PROG = None


def _build_program():
    import concourse.bacc as bacc
    import concourse.mybir as mybir
    import concourse.tile as tile

    F32 = mybir.dt.float32
    F16 = mybir.dt.float16
    F8 = mybir.dt.float8e4
    AF = mybir.ActivationFunctionType
    MUL = mybir.AluOpType.mult
    ADD = mybir.AluOpType.add
    DR = mybir.MatmulPerfMode.DoubleRow

    nc = bacc.Bacc("TRN2", target_bir_lowering=False, debug=False,
                   num_devices=NCORES)

    # x pre-tiled on host: [blk, kp, ko, t] (contiguous per block). All 16
    # k-tiles in fp16 (gate uses them all); tiles 8..15 additionally in e4m3
    # for the DoubleRow half of the qkv contraction.
    x16 = nc.dram_tensor("x16", [NBLK, P, KO, BLK], F16, kind="ExternalInput")
    x8 = nc.dram_tensor("x8", [NBLK, P, KO8, BLK], F8, kind="ExternalInput")
    # weights pre-tiled on host: [kp, ko, m] (m = 4 heads x 128)
    wq16 = nc.dram_tensor("wq16", [P, KO16, 512], F16, kind="ExternalInput")
    wk16 = nc.dram_tensor("wk16", [P, KO16, 512], F16, kind="ExternalInput")
    wv16 = nc.dram_tensor("wv16", [P, KO16, 512], F16, kind="ExternalInput")
    wq8 = nc.dram_tensor("wq8", [P, KO8, 512], F8, kind="ExternalInput")
    wk8 = nc.dram_tensor("wk8", [P, KO8, 512], F8, kind="ExternalInput")
    wv8 = nc.dram_tensor("wv8", [P, KO8, 512], F8, kind="ExternalInput")
    wg = nc.dram_tensor("wg", [P, KO, 512], F16, kind="ExternalInput")
    # w_out pre-tiled on host: [kp, kh(4 heads), n]
    wo = nc.dram_tensor("wo", [P, 4, HID], F16, kind="ExternalInput")
    ddT = nc.dram_tensor("ddT", [4, P, C], F32, kind="ExternalInput")
    qdec = nc.dram_tensor("qdec", [4, P, C], F16, kind="ExternalInput")
    kdec = nc.dram_tensor("kdec", [P, 4], F32, kind="ExternalInput")
    bdec = nc.dram_tensor("bdec", [P, 4], F32, kind="ExternalInput")
    idn = nc.dram_tensor("idn", [P, P], F16, kind="ExternalInput")
    out = nc.dram_tensor("out", [S, HID], F16, kind="ExternalOutput")

    with tile.TileContext(nc) as tc:
        with tc.tile_pool(name="const", bufs=1) as cpool, \
             tc.tile_pool(name="kvpool", bufs=1) as kvpool, \
             tc.tile_pool(name="xpool", bufs=2) as xpool, \
             tc.tile_pool(name="qkpool", bufs=2) as qkpool, \
             tc.tile_pool(name="apool", bufs=4) as apool, \
             tc.tile_pool(name="opool", bufs=3) as opool, \
             tc.tile_pool(name="pproj", bufs=2, space="PSUM") as pproj, \
             tc.tile_pool(name="ptr", bufs=1, space="PSUM") as ptr, \
             tc.tile_pool(name="psc", bufs=3, space="PSUM") as psc, \
             tc.tile_pool(name="pout", bufs=2, space="PSUM") as pout:

            def load_x(blk, defer_tail=False):
                # split the fp16 x DMA so the first matmuls can start before
                # the whole 2MB block lands (region-based deps); x8 right
                # after the fp16 head since the q DoubleRow half needs it.
                # defer_tail returns a closure for the gate-only fp16 tiles
                # so block 0 can slot weight DMAs ahead of them.
                xt16 = xpool.tile([P, KO, BLK], F16, tag="x16")
                nc.sync.dma_start(xt16[:, :KO16], x16.ap()[blk][:, :KO16])
                xt8 = xpool.tile([P, KO8, BLK], F8, tag="x8")
                nc.sync.dma_start(xt8[:], x8.ap()[blk])

                def tail():
                    nc.sync.dma_start(xt16[:, KO16:], x16.ap()[blk][:, KO16:])

                if defer_tail:
                    return (xt16, xt8), tail
                tail()
                return xt16, xt8

            # kv state double-buffered by chunk parity: the update writes
            # buffer c%2 while the PE still reads (c-1)%2 — no WAR stall
            kv_sb = kvpool.tile([P, 4, 2, P], F16)
            nc.vector.memset(kv_sb[:], 0.0)

            # DMA order matters on the in-order sync queue: q-proj operands
            # first so the PE can start ~10us in, everything else behind.
            ws = {}

            def loadw(nm, wd, dt_, kon):
                wt = cpool.tile([P, kon, 512], dt_, tag=nm)
                nc.sync.dma_start(wt[:], wd.ap())
                ws[nm] = wt

            loadw("wq16", wq16, F16, KO16)
            cur_x, x0_tail = load_x(0, defer_tail=True)
            loadw("wq8", wq8, F8, KO8)
            loadw("wk16", wk16, F16, KO16)
            loadw("wk8", wk8, F8, KO8)
            loadw("wv16", wv16, F16, KO16)
            loadw("wv8", wv8, F8, KO8)
            x0_tail()
            loadw("wg", wg, F16, KO)
            ident = cpool.tile([P, P], F16)
            nc.sync.dma_start(ident[:], idn.ap())
            bd_sb = cpool.tile([P, 4], F32)
            nc.sync.dma_start(bd_sb[:], bdec.ap())
            dd_sb = cpool.tile([P, 4, C], F32)
            qd_sb = cpool.tile([P, 4, C], F16)
            kd_sb = cpool.tile([P, 4], F32)
            nc.sync.dma_start(kd_sb[:], kdec.ap())
            for lh in range(4):
                nc.sync.dma_start(dd_sb[:, lh], ddT.ap()[lh])
                nc.sync.dma_start(qd_sb[:, lh], qdec.ap()[lh])
            wo_sb = cpool.tile([P, 4, HID], F16)
            nc.sync.dma_start(wo_sb[:], wo.ap())

            for blk in range(NBLK):
                t0 = blk * BLK
                xt16, xt8 = cur_x
                if blk + 1 < NBLK:
                    cur_x = load_x(blk + 1)

                # ---- projections ----
                qsb = qkpool.tile([P, 4, BLK], F16, tag="qsb")
                ksb = qkpool.tile([P, 4, BLK], F16, tag="ksb")
                vsb = qkpool.tile([P, 4, BLK], F16, tag="vsb")
                gsb = qkpool.tile([P, 4, BLK], F32, tag="gsb")

                def proj_qk(w16t, w8t, dst):
                    # dst[:, dt, :] = silu(sum_k w[k, dt*128+m] x[k, t]) / 8192
                    for dt in range(4):
                        ps = pproj.tile([P, BLK], F32, tag="proj")
                        msl = slice(dt * P, (dt + 1) * P)
                        for j in range(KO16):
                            nc.tensor.matmul(
                                ps[:], w16t[:, j, msl], xt16[:, j, :],
                                start=(j == 0), stop=False)
                        for th in range(2):
                            tsl = slice(th * 256, (th + 1) * 256)
                            for j in range(KO8 // 2):
                                nc.tensor.matmul(
                                    ps[:, tsl], w8t[:, 2 * j:2 * j + 2, msl],
                                    xt8[:, 2 * j:2 * j + 2, tsl],
                                    perf_mode=DR, start=False,
                                    stop=(j == KO8 // 2 - 1))
                        nc.scalar.activation(dst[:, dt, :], ps[:],
                                             AF.Silu, scale=1.0 / (SX * SW))

                proj_qk(ws["wq16"], ws["wq8"], qsb)
                proj_qk(ws["wk16"], ws["wk8"], ksb)
                # v in [token, dcol] layout
                for tt in range(4):
                    tsl = slice(tt * P, (tt + 1) * P)
                    ps = pproj.tile([P, BLK], F32, tag="proj")
                    for j in range(KO16):
                        nc.tensor.matmul(
                            ps[:], xt16[:, j, tsl], ws["wv16"][:, j, :],
                            start=(j == 0), stop=False)
                    for dh in range(2):
                        dsl = slice(dh * 256, (dh + 1) * 256)
                        for j in range(KO8 // 2):
                            nc.tensor.matmul(
                                ps[:, dsl], xt8[:, 2 * j:2 * j + 2, tsl],
                                ws["wv8"][:, 2 * j:2 * j + 2, dsl],
                                perf_mode=DR, start=False,
                                stop=(j == KO8 // 2 - 1))
                    nc.scalar.activation(vsb[:, tt, :], ps[:],
                                         AF.Silu, scale=1.0 / (SX * SW))
                # gate last (single sigmoid table load); all-fp16, x scaled 16
                for dt in range(4):
                    msl = slice(dt * P, (dt + 1) * P)
                    ps = pproj.tile([P, BLK], F32, tag="proj")
                    for j in range(KO):
                        nc.tensor.matmul(
                            ps[:], ws["wg"][:, j, msl], xt16[:, j, :],
                            start=(j == 0), stop=(j == KO - 1))
                    nc.scalar.activation(gsb[:, dt, :], ps[:],
                                         AF.Sigmoid, scale=1.0 / SX)

                # ---- attention: 4 chunks of 128 x 4 heads ----
                go_sb = apool.tile([P, 4, BLK], F16, tag="go", bufs=2)
                for ch in range(4):
                    co = ch * C
                    csl = slice(co, co + C)
                    first_chunk = (blk == 0 and ch == 0)
                    par = (blk * 4 + ch) % 2
                    for lh in range(4):
                        vs = vsb[:, ch, lh * P:(lh + 1) * P]  # [j, e]
                        # k transposed to [j, d]; k_decay applied afterwards
                        # as a per-partition scalar (decay depends on j only)
                        knp = ptr.tile([P, P], F16, tag="tr")
                        nc.tensor.transpose(knp[:], ksb[:, lh, csl], ident[:])
                        kn = apool.tile([P, P], F16, tag="kn", bufs=2)
                        nc.vector.tensor_scalar_mul(kn[:], knp[:],
                                                    kd_sb[:, lh:lh + 1])
                        # kv <- bdecay * kv + (k kdecay)^T v  -- issued FIRST:
                        # the chunk->chunk serial chain is tr->kn->up->stt
                        # (parity double-buffer makes this safe before the
                        # o-term reads the previous state below)
                        up = psc.tile([P, P], F32, tag="sc")
                        nc.tensor.matmul(up[:], kn[:], vs, start=True, stop=True)
                        nc.vector.scalar_tensor_tensor(
                            kv_sb[:, lh, par], kv_sb[:, lh, 1 - par],
                            bd_sb[:, lh:lh + 1], up[:], MUL, ADD)
                        # scoresT[j, i] = (k q^T) * decayT
                        st = psc.tile([P, C], F32, tag="sc")
                        nc.tensor.matmul(st[:], ksb[:, lh, csl],
                                         qsb[:, lh, csl], start=True, stop=True)
                        sm = apool.tile([P, C], F16, tag="sm", bufs=2)
                        nc.vector.tensor_tensor(sm[:], st[:], dd_sb[:, lh], MUL)
                        # oT[e, i] = v^T scoresT + kv^T (q * q_decay)
                        ot = psc.tile([P, C], F32, tag="sc")
                        nc.tensor.matmul(ot[:], vs, sm[:],
                                         start=True, stop=first_chunk)
                        if not first_chunk:
                            qdq = apool.tile([P, C], F16, tag="qdq", bufs=2)
                            nc.vector.tensor_tensor(qdq[:], qsb[:, lh, csl],
                                                    qd_sb[:, lh], MUL)
                            nc.tensor.matmul(ot[:], kv_sb[:, lh, 1 - par],
                                             qdq[:], start=False, stop=True)
                        nc.vector.tensor_tensor(go_sb[:, lh, csl], ot[:],
                                                gsb[:, lh, csl], MUL)

                    # ---- out projection for this chunk's tokens (mt == ch);
                    # partial over this core's 4 heads ----
                    mt = ch
                    ob = opool.tile([P, HID], F16, tag="ob")
                    msl = slice(mt * P, (mt + 1) * P)
                    for nt in range(4):
                        nsl = slice(nt * 512, (nt + 1) * 512)
                        op = pout.tile([P, 512], F32, tag="out")
                        for lh in range(4):
                            nc.tensor.matmul(
                                op[:], go_sb[:, lh, msl], wo_sb[:, lh, nsl],
                                start=(lh == 0), stop=(lh == 3))
                        if nt % 2 == 0:
                            nc.scalar.activation(ob[:, nsl], op[:], AF.Copy)
                        else:
                            nc.vector.tensor_copy(ob[:, nsl], op[:])
                        if nt == 1:
                            nc.sync.dma_start(
                                out.ap()[t0 + mt * P:t0 + (mt + 1) * P, :1024],
                                ob[:, :1024])
                    nc.sync.dma_start(
                        out.ap()[t0 + mt * P:t0 + (mt + 1) * P, 1024:],
                        ob[:, 1024:])

    nc.compile()
    return nc


def _get_program():
    global _PROG
    if _PROG is None:
        _PROG = _build_program()
    return _PROG


def _prep_shared(x):
    """Per-batch x tiles (shared by the 4 cores of a batch)."""
    import ml_dtypes
    E4 = ml_dtypes.float8_e4m3
    xs = []
    for b in range(B):
        # [HID, S] -> [ko, kp, blk, t] -> [blk, kp, ko, t]
        r = (np.ascontiguousarray(x[b].T).reshape(KO, P, NBLK, BLK) * SX)
        x16 = np.ascontiguousarray(r.transpose(2, 1, 0, 3)).astype(np.float16)
        x8 = np.ascontiguousarray(
            r[KO16:].transpose(2, 1, 0, 3)).astype(E4)
        xs.append((x16, x8))
    return xs


def _prep_core_inputs(xs, w_qkv, w_gate, w_out, slopes, core):
    import ml_dtypes
    E4 = ml_dtypes.float8_e4m3
    b, g = core // 4, core % 4
    h0 = 4 * g
    s = np.asarray(slopes, dtype=np.float32).reshape(H)[h0:h0 + 4]  # [4]
    x16, x8 = xs[b]

    def wtile(w2, scale):
        # [HID, 512] -> [kp, ko, m] split fp16/fp8 halves
        r = (w2.reshape(KO, P, 512) * scale).transpose(1, 0, 2)
        w16t = np.ascontiguousarray(r[:, :KO16]).astype(np.float16)
        w8t = np.ascontiguousarray(r[:, KO16:]).astype(E4)
        return w16t, w8t

    cq = slice(h0 * D, h0 * D + 512)
    wq16_c, wq8_c = wtile(w_qkv[:, cq], SW)
    wk16_c, wk8_c = wtile(w_qkv[:, 2048 + h0 * D: 2048 + h0 * D + 512], SW)
    wv16_c, wv8_c = wtile(w_qkv[:, 4096 + h0 * D: 4096 + h0 * D + 512], SW)
    wg_c = np.ascontiguousarray(
        w_gate[:, cq].reshape(KO, P, 512).transpose(1, 0, 2)).astype(np.float16)
    # [512, HID] -> [kp, kh, n]
    wo_c = np.ascontiguousarray(
        w_out[cq, :].reshape(4, P, HID).transpose(1, 0, 2)).astype(np.float16)

    pos = np.arange(C, dtype=np.float32)
    idx = pos[:, None] - pos[None, :]                      # [i, j] -> i - j
    ddT = np.empty((4, P, C), dtype=np.float32)
    qdec = np.empty((4, P, C), dtype=np.float16)
    kdec = np.empty((P, 4), dtype=np.float32)              # [j, lh]
    bdec = np.empty((P, 4), dtype=np.float32)
    for lh in range(4):
        sh = np.float64(s[lh])
        m = np.where(idx >= 0, np.exp(-sh * idx), 0.0)     # [i, j]
        ddT[lh] = m.T.astype(np.float32)                   # [j, i]
        qdec[lh] = np.broadcast_to(
            np.exp(-sh * (pos + 1.0)).astype(np.float16)[None, :], (P, C))
        kdec[:, lh] = np.exp(-sh * (C - 1.0 - pos)).astype(np.float32)
        bdec[:, lh] = np.float32(np.exp(-sh * C))

    return {
        "x16": x16, "x8": x8,
        "wq16": wq16_c, "wk16": wk16_c, "wv16": wv16_c,
        "wq8": wq8_c, "wk8": wk8_c, "wv8": wv8_c,
        "wg": wg_c, "wo": wo_c,
        "ddT": ddT, "qdec": qdec, "kdec": kdec,
        "bdec": np.ascontiguousarray(bdec),
        "idn": np.eye(P, dtype=np.float16),
    }


def kernel(x, w_qkv, w_gate, w_out, slopes, _trace=False, _result_holder=None):
    from concourse.bass_utils import run_bass_kernel_spmd

    x = np.asarray(x, dtype=np.float32)
    w_qkv = np.asarray(w_qkv, dtype=np.float32)
    w_gate = np.asarray(w_gate, dtype=np.float32)
    w_out = np.asarray(w_out, dtype=np.float32)

    nc = _get_program()
    xs = _prep_shared(x)
    in_maps = [_prep_core_inputs(xs, w_qkv, w_gate, w_out, slopes, c)
               for c in range(NCORES)]
    res = run_bass_kernel_spmd(nc, in_maps, core_ids=list(range(NCORES)),
                               trace=_trace)
    if _result_holder is not None:
        _result_holder.append(res)

    out = np.zeros((B, S, HID), dtype=np.float32)
    for c in range(NCORES):
        b = c // 4
        out[b] += res.results[c]["out"].astype(np.float32)
    return out


# revision 37
# speedup vs baseline: 1.2008x; 1.2008x over previous
"""MiniMaxText01 linear attention on 8 trn2 NeuronCores.

Sharding: core c -> batch b = c//4, head-quad g = c%4 (global heads 4g..4g+3).
Single merged pass over all 4 heads per core; out_proj is row-parallel so each
core emits one fp16 partial [S, HID]; the host sums 4 partials per batch.

All on-chip tensors live in transposed [feature, token] layout so every matmul
contraction sits on the partition dim. The qkv projection contracts K=2048 as
8 fp16 K-tiles + 8 e4m3 K-tiles (4 DoubleRow matmuls at 2x rate); operands are
pre-scaled by 16 (x) and 512 (w) on host and the product descaled for free via
the activation's scale argument. Gate/out projections and attention are fp16
with fp32 PSUM. Attention runs on 128-token chunks (mathematically identical
to the reference's 256 chunks, fewer FLOPs/token).
"""

import numpy as np

B, S, HID = 2, 4096, 2048
H, D = 16, 128
C = 128                # attention chunk
BLK = 512              # token block (4 chunks)
NBLK = S // BLK        # 8 blocks
KO = HID // 128        # 16 contraction subtiles
KO16 = 4               # k-tiles 0..3 in fp16
KO8 = 12               # k-tiles 4..15 in e4m3 (DoubleRow pairs)
NCORES = 8
P = 128
SX = 16.0              # x pre-scale (both halves)
SW = 512.0             # w_qkv/wv pre-scale (both halves)

_PROG = None


def _build_program():
    import concourse.bacc as bacc
    import concourse.mybir as mybir
    import concourse.tile as tile

    F32 = mybir.dt.float32
    F16 = mybir.dt.float16
    F8 = mybir.dt.float8e4
    AF = mybir.ActivationFunctionType
    MUL = mybir.AluOpType.mult
    ADD = mybir.AluOpType.add
    DR = mybir.MatmulPerfMode.DoubleRow

    nc = bacc.Bacc("TRN2", target_bir_lowering=False, debug=False,
                   num_devices=NCORES)

    # x pre-tiled on host: [blk, kp, ko, t] (contiguous per block). All 16
    # k-tiles in fp16 (gate uses them all); tiles 8..15 additionally in e4m3
    # for the DoubleRow half of the qkv contraction.
    x16 = nc.dram_tensor("x16", [NBLK, P, KO, BLK], F16, kind="ExternalInput")
    x8 = nc.dram_tensor("x8", [NBLK, P, KO8, BLK], F8, kind="ExternalInput")
    # weights pre-tiled on host: [kp, ko, m] (m = 4 heads x 128)
    wq16 = nc.dram_tensor("wq16", [P, KO16, 512], F16, kind="ExternalInput")
    wk16 = nc.dram_tensor("wk16", [P, KO16, 512], F16, kind="ExternalInput")
    wv16 = nc.dram_tensor("wv16", [P, KO16, 512], F16, kind="ExternalInput")
    wq8 = nc.dram_tensor("wq8", [P, KO8, 512], F8, kind="ExternalInput")
    wk8 = nc.dram_tensor("wk8", [P, KO8, 512], F8, kind="ExternalInput")
    wv8 = nc.dram_tensor("wv8", [P, KO8, 512], F8, kind="ExternalInput")
    wg = nc.dram_tensor("wg", [P, KO, 512], F16, kind="ExternalInput")
    # w_out pre-tiled on host: [kp, kh(4 heads), n]
    wo = nc.dram_tensor("wo", [P, 4, HID], F16, kind="ExternalInput")
    ddT = nc.dram_tensor("ddT", [4, P, C], F32, kind="ExternalInput")
    qdec = nc.dram_tensor("qdec", [4, P, C], F16, kind="ExternalInput")
    kdec = nc.dram_tensor("kdec", [P, 4], F32, kind="ExternalInput")
    bdec = nc.dram_tensor("bdec", [P, 4], F32, kind="ExternalInput")
    idn = nc.dram_tensor("idn", [P, P], F16, kind="ExternalInput")
    out = nc.dram_tensor("out", [S, HID], F16, kind="ExternalOutput")

    with tile.TileContext(nc) as tc:
        with tc.tile_pool(name="const", bufs=1) as cpool, \
             tc.tile_pool(name="kvpool", bufs=1) as kvpool, \
             tc.tile_pool(name="xpool", bufs=2) as xpool, \
             tc.tile_pool(name="qkpool", bufs=2) as qkpool, \
             tc.tile_pool(name="apool", bufs=4) as apool, \
             tc.tile_pool(name="opool", bufs=3) as opool, \
             tc.tile_pool(name="pproj", bufs=2, space="PSUM") as pproj, \
             tc.tile_pool(name="ptr", bufs=1, space="PSUM") as ptr, \
             tc.tile_pool(name="psc", bufs=3, space="PSUM") as psc, \
             tc.tile_pool(name="pout", bufs=2, space="PSUM") as pout:

            def load_x(blk, defer_tail=False):
                # split the fp16 x DMA so the first matmuls can start before
                # the whole 2MB block lands (region-based deps); x8 right
                # after the fp16 head since the q DoubleRow half needs it.
                # defer_tail returns a closure for the gate-only fp16 tiles
                # so block 0 can slot weight DMAs ahead of them.
                xt16 = xpool.tile([P, KO, BLK], F16, tag="x16")
                nc.sync.dma_start(xt16[:, :KO16], x16.ap()[blk][:, :KO16])
                xt8 = xpool.tile([P, KO8, BLK], F8, tag="x8")
                nc.sync.dma_start(xt8[:], x8.ap()[blk])

                def tail():
                    nc.sync.dma_start(xt16[:, KO16:], x16.ap()[blk][:, KO16:])

                if defer_tail:
                    return (xt16, xt8), tail
                tail()
                return xt16, xt8

            # kv state double-buffered by chunk parity: the update writes
            # buffer c%2 while the PE still reads (c-1)%2 — no WAR stall
            kv_sb = kvpool.tile([P, 4, 2, P], F16)
            nc.vector.memset(kv_sb[:], 0.0)

            # DMA order matters on the in-order sync queue: q-proj operands
            # first so the PE can start ~10us in, everything else behind.
            ws = {}

            def loadw(nm, wd, dt_, kon):
                wt = cpool.tile([P, kon, 512], dt_, tag=nm)
                nc.sync.dma_start(wt[:], wd.ap())
                ws[nm] = wt

            loadw("wq16", wq16, F16, KO16)
            cur_x, x0_tail = load_x(0, defer_tail=True)
            loadw("wq8", wq8, F8, KO8)
            loadw("wk16", wk16, F16, KO16)
            loadw("wk8", wk8, F8, KO8)
            loadw("wv16", wv16, F16, KO16)
            loadw("wv8", wv8, F8, KO8)
            x0_tail()
            loadw("wg", wg, F16, KO)
            ident = cpool.tile([P, P], F16)
            nc.sync.dma_start(ident[:], idn.ap())
            bd_sb = cpool.tile([P, 4], F32)
            nc.sync.dma_start(bd_sb[:], bdec.ap())
            dd_sb = cpool.tile([P, 4, C], F32)
            qd_sb = cpool.tile([P, 4, C], F16)
            kd_sb = cpool.tile([P, 4], F32)
            nc.sync.dma_start(kd_sb[:], kdec.ap())
            for lh in range(4):
                nc.sync.dma_start(dd_sb[:, lh], ddT.ap()[lh])
                nc.sync.dma_start(qd_sb[:, lh], qdec.ap()[lh])
            wo_sb = cpool.tile([P, 4, HID], F16)
            nc.sync.dma_start(wo_sb[:], wo.ap())

            for blk in range(NBLK):
                t0 = blk * BLK
                xt16, xt8 = cur_x
                if blk + 1 < NBLK:
                    cur_x = load_x(blk + 1)

                # ---- projections ----
                qsb = qkpool.tile([P, 4, BLK], F16, tag="qsb")
                ksb = qkpool.tile([P, 4, BLK], F16, tag="ksb")
                vsb = qkpool.tile([P, 4, BLK], F16, tag="vsb")
                gsb = qkpool.tile([P, 4, BLK], F32, tag="gsb")

                def proj_qk(w16t, w8t, dst):
                    # dst[:, dt, :] = silu(sum_k w[k, dt*128+m] x[k, t]) / 8192
                    for dt in range(4):
                        ps = pproj.tile([P, BLK], F32, tag="proj")
                        msl = slice(dt * P, (dt + 1) * P)
                        for j in range(KO16):
                            nc.tensor.matmul(
                                ps[:], w16t[:, j, msl], xt16[:, j, :],
                                start=(j == 0), stop=False)
                        for th in range(2):
                            tsl = slice(th * 256, (th + 1) * 256)
                            for j in range(KO8 // 2):
                                nc.tensor.matmul(
                                    ps[:, tsl], w8t[:, 2 * j:2 * j + 2, msl],
                                    xt8[:, 2 * j:2 * j + 2, tsl],
                                    perf_mode=DR, start=False,
                                    stop=(j == KO8 // 2 - 1))
                        nc.scalar.activation(dst[:, dt, :], ps[:],
                                             AF.Silu, scale=1.0 / (SX * SW))

                proj_qk(ws["wq16"], ws["wq8"], qsb)
                proj_qk(ws["wk16"], ws["wk8"], ksb)
                # v in [token, dcol] layout
                for tt in range(4):
                    tsl = slice(tt * P, (tt + 1) * P)
                    ps = pproj.tile([P, BLK], F32, tag="proj")
                    for j in range(KO16):
                        nc.tensor.matmul(
                            ps[:], xt16[:, j, tsl], ws["wv16"][:, j, :],
                            start=(j == 0), stop=False)
                    for dh in range(2):
                        dsl = slice(dh * 256, (dh + 1) * 256)
                        for j in range(KO8 // 2):
                            nc.tensor.matmul(
                                ps[:, dsl], xt8[:, 2 * j:2 * j + 2, tsl],
                                ws["wv8"][:, 2 * j:2 * j + 2, dsl],
                                perf_mode=DR, start=False,
                                stop=(j == KO8 // 2 - 1))
                    nc.scalar.activation(vsb[:, tt, :], ps[:],
                                         AF.Silu, scale=1.0 / (SX * SW))
                # gate last (single sigmoid table load); all-fp16, x scaled 16
                for dt in range(4):
                    msl = slice(dt * P, (dt + 1) * P)
                    ps = pproj.tile([P, BLK], F32, tag="proj")
                    for j in range(KO):
                        nc.tensor.matmul(
                            ps[:], ws["wg"][:, j, msl], xt16[:, j, :],
                            start=(j == 0), stop=(j == KO - 1))
                    nc.scalar.activation(gsb[:, dt, :], ps[:],
                                         AF.Sigmoid, scale=1.0 / SX)

                # ---- attention: 4 chunks of 128 x 4 heads ----
                go_sb = apool.tile([P, 4, BLK], F16, tag="go", bufs=2)
                for ch in range(4):
                    co = ch * C
                    csl = slice(co, co + C)
                    first_chunk = (blk == 0 and ch == 0)
                    par = (blk * 4 + ch) % 2
                    for lh in range(4):
                        vs = vsb[:, ch, lh * P:(lh + 1) * P]  # [j, e]
                        # k transposed to [j, d]; k_decay applied afterwards
                        # as a per-partition scalar (decay depends on j only)
                        knp = ptr.tile([P, P], F16, tag="tr")
                        nc.tensor.transpose(knp[:], ksb[:, lh, csl], ident[:])
                        kn = apool.tile([P, P], F16, tag="kn", bufs=2)
                        nc.vector.tensor_scalar_mul(kn[:], knp[:],
                                                    kd_sb[:, lh:lh + 1])
                        # kv update matmul early in the PE order (the
                        # chunk->chunk chain), but its state write (stt)
                        # stays LAST in the in-order vector queue
                        up = psc.tile([P, P], F32, tag="sc")
                        nc.tensor.matmul(up[:], kn[:], vs, start=True, stop=True)
                        # scoresT[j, i] = (k q^T) * decayT
                        st = psc.tile([P, C], F32, tag="sc")
                        nc.tensor.matmul(st[:], ksb[:, lh, csl],
                                         qsb[:, lh, csl], start=True, stop=True)
                        sm = apool.tile([P, C], F16, tag="sm", bufs=2)
                        nc.vector.tensor_tensor(sm[:], st[:], dd_sb[:, lh], MUL)
                        # oT[e, i] = v^T scoresT + kv^T (q * q_decay)
                        ot = psc.tile([P, C], F32, tag="sc")
                        nc.tensor.matmul(ot[:], vs, sm[:],
                                         start=True, stop=first_chunk)
                        if not first_chunk:
                            qdq = apool.tile([P, C], F16, tag="qdq", bufs=2)
                            nc.vector.tensor_tensor(qdq[:], qsb[:, lh, csl],
                                                    qd_sb[:, lh], MUL)
                            nc.tensor.matmul(ot[:], kv_sb[:, lh, 1 - par],
                                             qdq[:], start=False, stop=True)
                        nc.vector.tensor_tensor(go_sb[:, lh, csl], ot[:],
                                                gsb[:, lh, csl], MUL)
                        nc.vector.scalar_tensor_tensor(
                            kv_sb[:, lh, par], kv_sb[:, lh, 1 - par],
                            bd_sb[:, lh:lh + 1], up[:], MUL, ADD)

                    # ---- out projection for this chunk's tokens (mt == ch);
                    # partial over this core's 4 heads ----
                    mt = ch
                    ob = opool.tile([P, HID], F16, tag="ob")
                    msl = slice(mt * P, (mt + 1) * P)
                    for nt in range(4):
                        nsl = slice(nt * 512, (nt + 1) * 512)
                        op = pout.tile([P, 512], F32, tag="out")
                        for lh in range(4):
                            nc.tensor.matmul(
                                op[:], go_sb[:, lh, msl], wo_sb[:, lh, nsl],
                                start=(lh == 0), stop=(lh == 3))
                        if nt % 2 == 0:
                            nc.scalar.activation(ob[:, nsl], op[:], AF.Copy)
                        else:
                            nc.vector.tensor_copy(ob[:, nsl], op[:])
                        if nt == 1:
                            nc.sync.dma_start(
                                out.ap()[t0 + mt * P:t0 + (mt + 1) * P, :1024],
                                ob[:, :1024])
                    nc.sync.dma_start(
                        out.ap()[t0 + mt * P:t0 + (mt + 1) * P, 1024:],
                        ob[:, 1024:])

    nc.compile()
    return nc


def _get_program():
    global _PROG
    if _PROG is None:
        _PROG = _build_program()
    return _PROG


def _prep_shared(x):
    """Per-batch x tiles (shared by the 4 cores of a batch)."""
    import ml_dtypes
    E4 = ml_dtypes.float8_e4m3
    xs = []
    for b in range(B):
        # [HID, S] -> [ko, kp, blk, t] -> [blk, kp, ko, t]
        r = (np.ascontiguousarray(x[b].T).reshape(KO, P, NBLK, BLK) * SX)
        x16 = np.ascontiguousarray(r.transpose(2, 1, 0, 3)).astype(np.float16)
        x8 = np.ascontiguousarray(
            r[KO16:].transpose(2, 1, 0, 3)).astype(E4)
        xs.append((x16, x8))
    return xs


def _prep_core_inputs(xs, w_qkv, w_gate, w_out, slopes, core):
    import ml_dtypes
    E4 = ml_dtypes.float8_e4m3
    b, g = core // 4, core % 4
    h0 = 4 * g
    s = np.asarray(slopes, dtype=np.float32).reshape(H)[h0:h0 + 4]  # [4]
    x16, x8 = xs[b]

    def wtile(w2, scale):
        # [HID, 512] -> [kp, ko, m] split fp16/fp8 halves
        r = (w2.reshape(KO, P, 512) * scale).transpose(1, 0, 2)
        w16t = np.ascontiguousarray(r[:, :KO16]).astype(np.float16)
        w8t = np.ascontiguousarray(r[:, KO16:]).astype(E4)
        return w16t, w8t

    cq = slice(h0 * D, h0 * D + 512)
    wq16_c, wq8_c = wtile(w_qkv[:, cq], SW)
    wk16_c, wk8_c = wtile(w_qkv[:, 2048 + h0 * D: 2048 + h0 * D + 512], SW)
    wv16_c, wv8_c = wtile(w_qkv[:, 4096 + h0 * D: 4096 + h0 * D + 512], SW)
    wg_c = np.ascontiguousarray(
        w_gate[:, cq].reshape(KO, P, 512).transpose(1, 0, 2)).astype(np.float16)
    # [512, HID] -> [kp, kh, n]
    wo_c = np.ascontiguousarray(
        w_out[cq, :].reshape(4, P, HID).transpose(1, 0, 2)).astype(np.float16)

    pos = np.arange(C, dtype=np.float32)
    idx = pos[:, None] - pos[None, :]                      # [i, j] -> i - j
    ddT = np.empty((4, P, C), dtype=np.float32)
    qdec = np.empty((4, P, C), dtype=np.float16)
    kdec = np.empty((P, 4), dtype=np.float32)              # [j, lh]
    bdec = np.empty((P, 4), dtype=np.float32)
    for lh in range(4):
        sh = np.float64(s[lh])
        m = np.where(idx >= 0, np.exp(-sh * idx), 0.0)     # [i, j]
        ddT[lh] = m.T.astype(np.float32)                   # [j, i]
        qdec[lh] = np.broadcast_to(
            np.exp(-sh * (pos + 1.0)).astype(np.float16)[None, :], (P, C))
        kdec[:, lh] = np.exp(-sh * (C - 1.0 - pos)).astype(np.float32)
        bdec[:, lh] = np.float32(np.exp(-sh * C))

    return {
        "x16": x16, "x8": x8,
        "wq16": wq16_c, "wk16": wk16_c, "wv16": wv16_c,
        "wq8": wq8_c, "wk8": wk8_c, "wv8": wv8_c,
        "wg": wg_c, "wo": wo_c,
        "ddT": ddT, "qdec": qdec, "kdec": kdec,
        "bdec": np.ascontiguousarray(bdec),
        "idn": np.eye(P, dtype=np.float16),
    }


def kernel(x, w_qkv, w_gate, w_out, slopes, _trace=False, _result_holder=None):
    from concourse.bass_utils import run_bass_kernel_spmd

    x = np.asarray(x, dtype=np.float32)
    w_qkv = np.asarray(w_qkv, dtype=np.float32)
    w_gate = np.asarray(w_gate, dtype=np.float32)
    w_out = np.asarray(w_out, dtype=np.float32)

    nc = _get_program()
    xs = _prep_shared(x)
    in_maps = [_prep_core_inputs(xs, w_qkv, w_gate, w_out, slopes, c)
               for c in range(NCORES)]
    res = run_bass_kernel_spmd(nc, in_maps, core_ids=list(range(NCORES)),
                               trace=_trace)
    if _result_holder is not None:
        _result_holder.append(res)

    out = np.zeros((B, S, HID), dtype=np.float32)
    for c in range(NCORES):
        b = c // 4
        out[b] += res.results[c]["out"].astype(np.float32)
    return out


# revision 39
# speedup vs baseline: 1.2081x; 1.0060x over previous
"""MiniMaxText01 linear attention on 8 trn2 NeuronCores.

Sharding: core c -> batch b = c//4, head-quad g = c%4 (global heads 4g..4g+3).
Single merged pass over all 4 heads per core; out_proj is row-parallel so each
core emits one fp16 partial [S, HID]; the host sums 4 partials per batch.

All on-chip tensors live in transposed [feature, token] layout so every matmul
contraction sits on the partition dim. The qkv projection contracts K=2048 as
8 fp16 K-tiles + 8 e4m3 K-tiles (4 DoubleRow matmuls at 2x rate); operands are
pre-scaled by 16 (x) and 512 (w) on host and the product descaled for free via
the activation's scale argument. Gate/out projections and attention are fp16
with fp32 PSUM. Attention runs on 128-token chunks (mathematically identical
to the reference's 256 chunks, fewer FLOPs/token).
"""

import numpy as np

B, S, HID = 2, 4096, 2048
H, D = 16, 128
C = 128                # attention chunk
BLK = 512              # token block (4 chunks)
NBLK = S // BLK        # 8 blocks
KO = HID // 128        # 16 contraction subtiles
KO16 = 4               # k-tiles 0..3 in fp16
KO8 = 12               # k-tiles 4..15 in e4m3 (DoubleRow pairs)
NCORES = 8
P = 128
SX = 16.0              # x pre-scale (both halves)
SW = 512.0             # w_qkv/wv pre-scale (both halves)

_PROG = None


def _build_program():
    import concourse.bacc as bacc
    import concourse.mybir as mybir
    import concourse.tile as tile

    F32 = mybir.dt.float32
    F16 = mybir.dt.float16
    F8 = mybir.dt.float8e4
    AF = mybir.ActivationFunctionType
    MUL = mybir.AluOpType.mult
    ADD = mybir.AluOpType.add
    DR = mybir.MatmulPerfMode.DoubleRow

    nc = bacc.Bacc("TRN2", target_bir_lowering=False, debug=False,
                   num_devices=NCORES)

    # x pre-tiled on host: [blk, kp, ko, t] (contiguous per block). All 16
    # k-tiles in fp16 (gate uses them all); tiles 8..15 additionally in e4m3
    # for the DoubleRow half of the qkv contraction.
    x16 = nc.dram_tensor("x16", [NBLK, P, KO, BLK], F16, kind="ExternalInput")
    x8 = nc.dram_tensor("x8", [NBLK, P, KO8, BLK], F8, kind="ExternalInput")
    # weights pre-tiled on host: [kp, ko, m] (m = 4 heads x 128)
    wq16 = nc.dram_tensor("wq16", [P, KO16, 512], F16, kind="ExternalInput")
    wk16 = nc.dram_tensor("wk16", [P, KO16, 512], F16, kind="ExternalInput")
    wv16 = nc.dram_tensor("wv16", [P, KO16, 512], F16, kind="ExternalInput")
    wq8 = nc.dram_tensor("wq8", [P, KO8, 512], F8, kind="ExternalInput")
    wk8 = nc.dram_tensor("wk8", [P, KO8, 512], F8, kind="ExternalInput")
    wv8 = nc.dram_tensor("wv8", [P, KO8, 512], F8, kind="ExternalInput")
    wg = nc.dram_tensor("wg", [P, KO, 512], F16, kind="ExternalInput")
    # w_out pre-tiled on host: [kp, kh(4 heads), n]
    wo = nc.dram_tensor("wo", [P, 4, HID], F16, kind="ExternalInput")
    ddT = nc.dram_tensor("ddT", [4, P, C], F32, kind="ExternalInput")
    qdec = nc.dram_tensor("qdec", [4, P, C], F16, kind="ExternalInput")
    kdec = nc.dram_tensor("kdec", [P, 4], F32, kind="ExternalInput")
    bdec = nc.dram_tensor("bdec", [P, 4], F32, kind="ExternalInput")
    idn = nc.dram_tensor("idn", [P, P], F16, kind="ExternalInput")
    out = nc.dram_tensor("out", [S, HID], F16, kind="ExternalOutput")

    with tile.TileContext(nc) as tc:
        with tc.tile_pool(name="const", bufs=1) as cpool, \
             tc.tile_pool(name="kvpool", bufs=1) as kvpool, \
             tc.tile_pool(name="xpool", bufs=2) as xpool, \
             tc.tile_pool(name="qkpool", bufs=2) as qkpool, \
             tc.tile_pool(name="apool", bufs=4) as apool, \
             tc.tile_pool(name="opool", bufs=3) as opool, \
             tc.tile_pool(name="pproj", bufs=2, space="PSUM") as pproj, \
             tc.tile_pool(name="ptr", bufs=1, space="PSUM") as ptr, \
             tc.tile_pool(name="psc", bufs=3, space="PSUM") as psc, \
             tc.tile_pool(name="pout", bufs=2, space="PSUM") as pout:

            def load_x(blk, defer_tail=False):
                # split the fp16 x DMA so the first matmuls can start before
                # the whole 2MB block lands (region-based deps); x8 right
                # after the fp16 head since the q DoubleRow half needs it.
                # defer_tail returns a closure for the gate-only fp16 tiles
                # so block 0 can slot weight DMAs ahead of them.
                xt16 = xpool.tile([P, KO, BLK], F16, tag="x16")
                nc.sync.dma_start(xt16[:, :KO16], x16.ap()[blk][:, :KO16])
                xt8 = xpool.tile([P, KO8, BLK], F8, tag="x8")
                nc.sync.dma_start(xt8[:], x8.ap()[blk])

                def tail():
                    nc.sync.dma_start(xt16[:, KO16:], x16.ap()[blk][:, KO16:])

                if defer_tail:
                    return (xt16, xt8), tail
                tail()
                return xt16, xt8

            # kv state double-buffered by chunk parity: the update writes
            # buffer c%2 while the PE still reads (c-1)%2 — no WAR stall
            kv_sb = kvpool.tile([P, 4, 2, P], F16)
            nc.vector.memset(kv_sb[:], 0.0)

            # DMA order matters on the in-order sync queue: q-proj operands
            # first so the PE can start ~10us in, everything else behind.
            ws = {}

            def loadw(nm, wd, dt_, kon):
                wt = cpool.tile([P, kon, 512], dt_, tag=nm)
                nc.sync.dma_start(wt[:], wd.ap())
                ws[nm] = wt

            loadw("wq16", wq16, F16, KO16)
            cur_x, x0_tail = load_x(0, defer_tail=True)
            loadw("wq8", wq8, F8, KO8)
            loadw("wk16", wk16, F16, KO16)
            loadw("wk8", wk8, F8, KO8)
            loadw("wv16", wv16, F16, KO16)
            loadw("wv8", wv8, F8, KO8)
            x0_tail()
            loadw("wg", wg, F16, KO)
            ident = cpool.tile([P, P], F16)
            nc.sync.dma_start(ident[:], idn.ap())
            bd_sb = cpool.tile([P, 4], F32)
            nc.sync.dma_start(bd_sb[:], bdec.ap())
            dd_sb = cpool.tile([P, 4, C], F32)
            qd_sb = cpool.tile([P, 4, C], F16)
            kd_sb = cpool.tile([P, 4], F32)
            nc.sync.dma_start(kd_sb[:], kdec.ap())
            for lh in range(4):
                nc.sync.dma_start(dd_sb[:, lh], ddT.ap()[lh])
                nc.sync.dma_start(qd_sb[:, lh], qdec.ap()[lh])
            wo_sb = cpool.tile([P, 4, HID], F16)
            nc.sync.dma_start(wo_sb[:], wo.ap())

            for blk in range(NBLK):
                t0 = blk * BLK
                xt16, xt8 = cur_x
                if blk + 1 < NBLK:
                    cur_x = load_x(blk + 1)

                # ---- projections ----
                qsb = qkpool.tile([P, 4, BLK], F16, tag="qsb")
                ksb = qkpool.tile([P, 4, BLK], F16, tag="ksb")
                vsb = qkpool.tile([P, 4, BLK], F16, tag="vsb")
                gsb = qkpool.tile([P, 4, BLK], F32, tag="gsb")

                def proj_qk(w16t, w8t, dst):
                    # dst[:, dt, :] = silu(sum_k w[k, dt*128+m] x[k, t]) / 8192
                    for dt in range(4):
                        ps = pproj.tile([P, BLK], F32, tag="proj")
                        msl = slice(dt * P, (dt + 1) * P)
                        for j in range(KO16):
                            nc.tensor.matmul(
                                ps[:], w16t[:, j, msl], xt16[:, j, :],
                                start=(j == 0), stop=False)
                        for th in range(2):
                            tsl = slice(th * 256, (th + 1) * 256)
                            for j in range(KO8 // 2):
                                nc.tensor.matmul(
                                    ps[:, tsl], w8t[:, 2 * j:2 * j + 2, msl],
                                    xt8[:, 2 * j:2 * j + 2, tsl],
                                    perf_mode=DR, start=False,
                                    stop=(j == KO8 // 2 - 1))
                        nc.scalar.activation(dst[:, dt, :], ps[:],
                                             AF.Silu, scale=1.0 / (SX * SW))

                proj_qk(ws["wq16"], ws["wq8"], qsb)
                proj_qk(ws["wk16"], ws["wk8"], ksb)
                # v in [token, dcol] layout
                for tt in range(4):
                    tsl = slice(tt * P, (tt + 1) * P)
                    ps = pproj.tile([P, BLK], F32, tag="proj")
                    for j in range(KO16):
                        nc.tensor.matmul(
                            ps[:], xt16[:, j, tsl], ws["wv16"][:, j, :],
                            start=(j == 0), stop=False)
                    for dh in range(2):
                        dsl = slice(dh * 256, (dh + 1) * 256)
                        for j in range(KO8 // 2):
                            nc.tensor.matmul(
                                ps[:, dsl], xt8[:, 2 * j:2 * j + 2, tsl],
                                ws["wv8"][:, 2 * j:2 * j + 2, dsl],
                                perf_mode=DR, start=False,
                                stop=(j == KO8 // 2 - 1))
                    nc.scalar.activation(vsb[:, tt, :], ps[:],
                                         AF.Silu, scale=1.0 / (SX * SW))
                # gate last (single sigmoid table load); all-fp16, x scaled 16
                def emit_gate(th0, th1):
                    tsl = slice(th0 * 256, th1 * 256)
                    for dt in range(4):
                        msl = slice(dt * P, (dt + 1) * P)
                        ps = pproj.tile([P, BLK], F32, tag="proj")
                        for j in range(KO):
                            nc.tensor.matmul(
                                ps[:, tsl], ws["wg"][:, j, msl],
                                xt16[:, j, tsl],
                                start=(j == 0), stop=(j == KO - 1))
                        nc.scalar.activation(gsb[:, dt, tsl], ps[:, tsl],
                                             AF.Sigmoid, scale=1.0 / SX)

                # ---- attention: 4 chunks of 128 x 4 heads ----
                go_sb = apool.tile([P, 4, BLK], F16, tag="go", bufs=2)

                def do_chunk(ch):
                    co = ch * C
                    csl = slice(co, co + C)
                    first_chunk = (blk == 0 and ch == 0)
                    par = (blk * 4 + ch) % 2
                    for lh in range(4):
                        vs = vsb[:, ch, lh * P:(lh + 1) * P]  # [j, e]
                        # k transposed to [j, d]; k_decay applied afterwards
                        # as a per-partition scalar (decay depends on j only)
                        knp = ptr.tile([P, P], F16, tag="tr")
                        nc.tensor.transpose(knp[:], ksb[:, lh, csl], ident[:])
                        kn = apool.tile([P, P], F16, tag="kn", bufs=2)
                        nc.vector.tensor_scalar_mul(kn[:], knp[:],
                                                    kd_sb[:, lh:lh + 1])
                        # kv update matmul early in the PE order (the
                        # chunk->chunk chain), but its state write (stt)
                        # stays LAST in the in-order vector queue
                        up = psc.tile([P, P], F32, tag="sc")
                        nc.tensor.matmul(up[:], kn[:], vs, start=True, stop=True)
                        # scoresT[j, i] = (k q^T) * decayT
                        st = psc.tile([P, C], F32, tag="sc")
                        nc.tensor.matmul(st[:], ksb[:, lh, csl],
                                         qsb[:, lh, csl], start=True, stop=True)
                        sm = apool.tile([P, C], F16, tag="sm", bufs=2)
                        nc.vector.tensor_tensor(sm[:], st[:], dd_sb[:, lh], MUL)
                        # oT[e, i] = v^T scoresT + kv^T (q * q_decay)
                        ot = psc.tile([P, C], F32, tag="sc")
                        nc.tensor.matmul(ot[:], vs, sm[:],
                                         start=True, stop=first_chunk)
                        if not first_chunk:
                            qdq = apool.tile([P, C], F16, tag="qdq", bufs=2)
                            nc.vector.tensor_tensor(qdq[:], qsb[:, lh, csl],
                                                    qd_sb[:, lh], MUL)
                            nc.tensor.matmul(ot[:], kv_sb[:, lh, 1 - par],
                                             qdq[:], start=False, stop=True)
                        nc.vector.tensor_tensor(go_sb[:, lh, csl], ot[:],
                                                gsb[:, lh, csl], MUL)
                        nc.vector.scalar_tensor_tensor(
                            kv_sb[:, lh, par], kv_sb[:, lh, 1 - par],
                            bd_sb[:, lh:lh + 1], up[:], MUL, ADD)

                    # ---- out projection for this chunk's tokens (mt == ch);
                    # partial over this core's 4 heads ----
                    mt = ch
                    ob = opool.tile([P, HID], F16, tag="ob")
                    msl = slice(mt * P, (mt + 1) * P)
                    for nt in range(4):
                        nsl = slice(nt * 512, (nt + 1) * 512)
                        op = pout.tile([P, 512], F32, tag="out")
                        for lh in range(4):
                            nc.tensor.matmul(
                                op[:], go_sb[:, lh, msl], wo_sb[:, lh, nsl],
                                start=(lh == 0), stop=(lh == 3))
                        if nt % 2 == 0:
                            nc.scalar.activation(ob[:, nsl], op[:], AF.Copy)
                        else:
                            nc.vector.tensor_copy(ob[:, nsl], op[:])
                        if nt == 1:
                            nc.sync.dma_start(
                                out.ap()[t0 + mt * P:t0 + (mt + 1) * P, :1024],
                                ob[:, :1024])
                    nc.sync.dma_start(
                        out.ap()[t0 + mt * P:t0 + (mt + 1) * P, 1024:],
                        ob[:, 1024:])

                if blk < NBLK - 1:
                    emit_gate(0, 2)
                    for ch in range(4):
                        do_chunk(ch)
                else:
                    # last block: no next-block projections to fill attention
                    # chain stalls, so feed the PE the second gate half there
                    emit_gate(0, 1)
                    do_chunk(0)
                    do_chunk(1)
                    emit_gate(1, 2)
                    do_chunk(2)
                    do_chunk(3)

    nc.compile()
    return nc


def _get_program():
    global _PROG
    if _PROG is None:
        _PROG = _build_program()
    return _PROG


def _prep_shared(x):
    """Per-batch x tiles (shared by the 4 cores of a batch)."""
    import ml_dtypes
    E4 = ml_dtypes.float8_e4m3
    xs = []
    for b in range(B):
        # [HID, S] -> [ko, kp, blk, t] -> [blk, kp, ko, t]
        r = (np.ascontiguousarray(x[b].T).reshape(KO, P, NBLK, BLK) * SX)
        x16 = np.ascontiguousarray(r.transpose(2, 1, 0, 3)).astype(np.float16)
        x8 = np.ascontiguousarray(
            r[KO16:].transpose(2, 1, 0, 3)).astype(E4)
        xs.append((x16, x8))
    return xs


def _prep_core_inputs(xs, w_qkv, w_gate, w_out, slopes, core):
    import ml_dtypes
    E4 = ml_dtypes.float8_e4m3
    b, g = core // 4, core % 4
    h0 = 4 * g
    s = np.asarray(slopes, dtype=np.float32).reshape(H)[h0:h0 + 4]  # [4]
    x16, x8 = xs[b]

    def wtile(w2, scale):
        # [HID, 512] -> [kp, ko, m] split fp16/fp8 halves
        r = (w2.reshape(KO, P, 512) * scale).transpose(1, 0, 2)
        w16t = np.ascontiguousarray(r[:, :KO16]).astype(np.float16)
        w8t = np.ascontiguousarray(r[:, KO16:]).astype(E4)
        return w16t, w8t

    cq = slice(h0 * D, h0 * D + 512)
    wq16_c, wq8_c = wtile(w_qkv[:, cq], SW)
    wk16_c, wk8_c = wtile(w_qkv[:, 2048 + h0 * D: 2048 + h0 * D + 512], SW)
    wv16_c, wv8_c = wtile(w_qkv[:, 4096 + h0 * D: 4096 + h0 * D + 512], SW)
    wg_c = np.ascontiguousarray(
        w_gate[:, cq].reshape(KO, P, 512).transpose(1, 0, 2)).astype(np.float16)
    # [512, HID] -> [kp, kh, n]
    wo_c = np.ascontiguousarray(
        w_out[cq, :].reshape(4, P, HID).transpose(1, 0, 2)).astype(np.float16)

    pos = np.arange(C, dtype=np.float32)
    idx = pos[:, None] - pos[None, :]                      # [i, j] -> i - j
    ddT = np.empty((4, P, C), dtype=np.float32)
    qdec = np.empty((4, P, C), dtype=np.float16)
    kdec = np.empty((P, 4), dtype=np.float32)              # [j, lh]
    bdec = np.empty((P, 4), dtype=np.float32)
    for lh in range(4):
        sh = np.float64(s[lh])
        m = np.where(idx >= 0, np.exp(-sh * idx), 0.0)     # [i, j]
        ddT[lh] = m.T.astype(np.float32)                   # [j, i]
        qdec[lh] = np.broadcast_to(
            np.exp(-sh * (pos + 1.0)).astype(np.float16)[None, :], (P, C))
        kdec[:, lh] = np.exp(-sh * (C - 1.0 - pos)).astype(np.float32)
        bdec[:, lh] = np.float32(np.exp(-sh * C))

    return {
        "x16": x16, "x8": x8,
        "wq16": wq16_c, "wk16": wk16_c, "wv16": wv16_c,
        "wq8": wq8_c, "wk8": wk8_c, "wv8": wv8_c,
        "wg": wg_c, "wo": wo_c,
        "ddT": ddT, "qdec": qdec, "kdec": kdec,
        "bdec": np.ascontiguousarray(bdec),
        "idn": np.eye(P, dtype=np.float16),
    }


def kernel(x, w_qkv, w_gate, w_out, slopes, _trace=False, _result_holder=None):
    from concourse.bass_utils import run_bass_kernel_spmd

    x = np.asarray(x, dtype=np.float32)
    w_qkv = np.asarray(w_qkv, dtype=np.float32)
    w_gate = np.asarray(w_gate, dtype=np.float32)
    w_out = np.asarray(w_out, dtype=np.float32)

    nc = _get_program()
    xs = _prep_shared(x)
    in_maps = [_prep_core_inputs(xs, w_qkv, w_gate, w_out, slopes, c)
               for c in range(NCORES)]
    res = run_bass_kernel_spmd(nc, in_maps, core_ids=list(range(NCORES)),
                               trace=_trace)
    if _result_holder is not None:
        _result_holder.append(res)

    out = np.zeros((B, S, HID), dtype=np.float32)
    for c in range(NCORES):
        b = c // 4
        out[b] += res.results[c]["out"].astype(np.float32)
    return out
